# revision 41
# baseline (speedup 1.0000x reference)
"""Multi-head attention (B=4, S=2048, D=1024, H=16) on 8 trn2 NeuronCores.

Sharding: data-parallel over batch (4) x tensor-parallel over head halves (2)
-> 8 cores.  Each core handles one (batch, 8-head half): projections,
attention, output projection; host sums the two half partials per batch.

Per-core design (all activations fp16, fp32 PSUM accumulate):
  - heads processed in PAIRS (even head A on PE rows/cols 0-63, odd head B
    on 64-127):
      QK^T: row-tiled pair of K=64 matmuls (tile_position (0,0)/(64,0)) ->
            scoresT [128 keys, 512q(A) | 512q(B)] in one PSUM sp tile
      exp:  ONE scalar-engine activation per (pair, key-tile) over
            [128, 1024] (both heads) -> E fp16 in SBUF
      PV:   col-tiled pair (tile_position (0,0)/(0,64)) accumulating into a
            single [128, 512] PSUM bank (A rows 0-63, B rows 64-127)
      denom: DVE accumulates E_sum over key tiles; two M=1 ones-matmuls
            (col positions 0 / 32) reduce over the 128 key partitions
  - projections for later head-pairs and the output projection are emitted
    interleaved into the attention periods, filling TensorE while the
    scalar engine (exp) paces the loop.
1/sqrt(head_dim) folded into wq on host.  Mask-supporting variant built
lazily if a nonzero mask is ever passed.
"""

import sys

for _p in ("/opt/trn_rl_repo",):
    if _p not in sys.path:
        sys.path.insert(0, _p)

from collections import deque
from contextlib import ExitStack

import numpy as np

import concourse.bass as bass
import concourse.tile as tile
from concourse import bacc, mybir
from concourse.bass_utils import run_bass_kernel_spmd

# problem constants (per core)
S = 2048          # sequence length
D = 1024          # model dim
DL = 512          # local (sharded) dim = 8 heads * 64
HL = 8            # local heads
HPN = 4           # head pairs
HD = 64           # head dim
P = 128           # partitions
CT = D // P       # contraction tiles for projections (8)
KT = S // P       # key tiles (16)
QC = 512          # q chunk
QCN = S // QC     # 4
F16 = mybir.dt.float16
F32 = mybir.dt.float32
AF = mybir.ActivationFunctionType
ALU = mybir.AluOpType


def build_program(s=S, with_mask=False, dump=False):
    nc = bacc.Bacc("TRN2", target_bir_lowering=False, debug=False, num_devices=8)

    qd = nc.dram_tensor("q", [s, D], F16, kind="ExternalInput").ap()
    kd = nc.dram_tensor("k", [s, D], F16, kind="ExternalInput").ap()
    vd = nc.dram_tensor("v", [s, D], F16, kind="ExternalInput").ap()
    wqd = nc.dram_tensor("wq", [D, DL], F16, kind="ExternalInput").ap()
    wkd = nc.dram_tensor("wk", [D, DL], F16, kind="ExternalInput").ap()
    wvd = nc.dram_tensor("wv", [D, DL], F16, kind="ExternalInput").ap()
    wod = nc.dram_tensor("wo", [DL, D], F16, kind="ExternalInput").ap()
    maskd = None
    if with_mask:
        maskd = nc.dram_tensor("maskT", [s, s], F32, kind="ExternalInput").ap()
    outd = nc.dram_tensor("out", [s, D], F32, kind="ExternalOutput").ap()

    kt_n = s // P
    qcn = s // QC

    with tile.TileContext(nc) as tc, ExitStack() as ctx:
        # ---------------- persistent SBUF ----------------
        cpool = ctx.enter_context(tc.tile_pool(name="const", bufs=1))
        wo_sb = cpool.tile([P, (DL // P) * D], F16)   # [128, 4*1024] dchunks
        xq_sb = cpool.tile([P, (DL // P) * s], F16)   # xqT: 4 dchunks x [128, s]
        xk_sb = cpool.tile([P, (DL // P) * s], F16)
        ao_sb = cpool.tile([P, (DL // P) * s], F16)   # attn_outT
        xv_sb = cpool.tile([P, kt_n * DL], F16)       # [keys 128, kt * (h*64)]
        ones_sb = cpool.tile([P, 1], F16)
        wq_sb = cpool.tile([P, CT * DL], F16)
        wk_sb = cpool.tile([P, CT * DL], F16)
        wv_sb = cpool.tile([P, CT * DL], F16)

        nc.vector.memset(ones_sb[:], 1.0)

        # transposed q/k activations: 8 contraction-dim chunks each, persistent
        qkt_pool = ctx.enter_context(tc.tile_pool(name="qkT", bufs=1))
        # PSUM pools: sp 2x2 banks + o 2x1 banks + scratch 2x1 banks = 8
        sp_pool = ctx.enter_context(tc.tile_pool(name="spsum", bufs=2, space="PSUM"))
        o_pool = ctx.enter_context(tc.tile_pool(name="opsum", bufs=2, space="PSUM"))
        ps_pool = ctx.enter_context(tc.tile_pool(name="pscratch", bufs=2, space="PSUM"))
        e_pool = ctx.enter_context(tc.tile_pool(name="exp", bufs=3))
        es_pool = ctx.enter_context(tc.tile_pool(name="esum", bufs=2))
        n_pool = ctx.enter_context(tc.tile_pool(name="norm", bufs=1))
        ob_pool = ctx.enter_context(tc.tile_pool(name="outsb", bufs=2))
        mt_pool = ctx.enter_context(tc.tile_pool(name="mask", bufs=2)) if with_mask else None

        # transposed q/k: single giant tiles, qT[:, ct*s + j] = q[j, ct*128 + p]
        qT = qkt_pool.tile([P, CT * s], F16)
        kT = qkt_pool.tile([P, CT * s], F16)

        def qk_chain(which, c, n0):
            """Project q or k: output d-chunk c, seq cols [n0*512, +512) -> x_sb."""
            tT, w_sb, x_sb = (qT, wq_sb, xq_sb) if which == "q" else (kT, wk_sb, xk_sb)
            ps = ps_pool.tile([P, QC], F32, tag="ps")
            for ct2 in range(CT):
                nc.tensor.matmul(
                    ps[:],
                    lhsT=w_sb[:, ct2 * DL + c * P: ct2 * DL + (c + 1) * P],
                    rhs=tT[:, ct2 * s + n0 * QC: ct2 * s + (n0 + 1) * QC],
                    start=(ct2 == 0), stop=(ct2 == CT - 1))
            nc.vector.tensor_copy(x_sb[:, c * s + n0 * QC: c * s + (n0 + 1) * QC], ps[:])

        def v_chain(vT, sc, st):
            """Project v: key-tile kt = sc*4+st -> xv_sb slice [128, 512]."""
            kt = sc * 4 + st
            ps = ps_pool.tile([P, QC], F32, tag="ps")
            for ct2 in range(CT):
                nc.tensor.matmul(
                    ps[:],
                    lhsT=vT[:, ct2 * QC + st * P: ct2 * QC + (st + 1) * P],
                    rhs=wv_sb[:, ct2 * DL:(ct2 + 1) * DL],
                    start=(ct2 == 0), stop=(ct2 == CT - 1))
            nc.vector.tensor_copy(xv_sb[:, kt * DL:(kt + 1) * DL], ps[:])

        def outproj_chain(qc, qt, dh):
            """Output projection for q-tile [qc*512+qt*128, +128), D cols [dh*512,+512)."""
            q0 = qc * QC + qt * P
            ps = ps_pool.tile([P, QC], F32, tag="ps")
            for dc in range(DL // P):
                nc.tensor.matmul(
                    ps[:],
                    lhsT=ao_sb[:, dc * s + q0: dc * s + q0 + P],
                    rhs=wo_sb[:, dc * D + dh * QC: dc * D + (dh + 1) * QC],
                    start=(dc == 0), stop=(dc == DL // P - 1))
            ob = ob_pool.tile([P, QC], F32, tag="ob")
            nc.vector.tensor_copy(ob[:], ps[:])
            nc.sync.dma_start(outd[q0:q0 + P, dh * QC:(dh + 1) * QC], ob[:])

        # ---------------- lead-in ----------------
        # 3-D-output transposes split by seq range so projection chains can
        # start as soon as their piece lands (qT3d[:, c, j] = x[j, c*128+p]).
        qT3 = qT[:].rearrange("p (c j) -> p c j", c=CT)
        kT3 = kT[:].rearrange("p (c j) -> p c j", c=CT)
        extra = deque()
        vts = []
        v_pool = ctx.enter_context(tc.tile_pool(name="vT", bufs=3))

        def tp_k(n0):
            j0 = n0 * QC
            nc.sync.dma_start_transpose(kT3[:, :, j0:j0 + QC], kd[j0:j0 + QC, 0:D])

        def tp_q(n0):
            j0 = n0 * QC
            nc.sync.dma_start_transpose(qT3[:, :, j0:j0 + QC], qd[j0:j0 + QC, 0:D])

        def tp_v(sc):
            vT = v_pool.tile([P, CT * QC], F16, tag="vT", bufs=3, name=f"vT{sc}")
            s0 = sc * QC
            nc.sync.dma_start_transpose(
                vT[:].rearrange("p (c j) -> p c j", c=CT), vd[s0:s0 + QC, 0:D])
            vts.append(vT)

        # weights first (fast, land before any consumer), then transposes in
        # consumption order.  Never issue fast DMAs after slow ones on the
        # shared queue semaphores: the completion counters conflate, and a
        # later fast DMA's increments release a slow transpose's consumers
        # before its data lands (observed on HW).
        for ct2 in range(CT):
            nc.sync.dma_start(wk_sb[:, ct2 * DL:(ct2 + 1) * DL], wkd[ct2 * P:(ct2 + 1) * P, :])
            nc.sync.dma_start(wq_sb[:, ct2 * DL:(ct2 + 1) * DL], wqd[ct2 * P:(ct2 + 1) * P, :])
        for ct2 in range(CT):
            nc.sync.dma_start(wv_sb[:, ct2 * DL:(ct2 + 1) * DL], wvd[ct2 * P:(ct2 + 1) * P, :])
        for dc in range(DL // P):
            nc.sync.dma_start(wo_sb[:, dc * D:(dc + 1) * D], wod[dc * P:(dc + 1) * P, :])
        tp_k(0); tp_q(0); tp_v(0)
        for n0 in range(1, s // QC):
            tp_k(n0); tp_v(n0)
        for n0 in range(1, s // QC):
            tp_q(n0)

        # minimal lead chains: first k/q/v pieces; the rest just-in-time
        qk_chain("k", 0, 0)
        qk_chain("q", 0, 0)
        for st in range(QC // P):
            v_chain(vts[0], 0, st)
        spp = QC // P  # key tiles per seq piece; deadlines carry >=4-period
        # margin -- a chain drained right before its consumer NaNs on HW
        for n0 in range(1, s // QC):
            extra.append((spp * n0 - 5, lambda n0=n0: qk_chain("k", 0, n0)))
            for st in range(QC // P):
                extra.append((spp * n0 + st - 3, lambda sc=n0, st=st: v_chain(vts[sc], sc, st)))
        for n0 in range(1, s // QC):
            extra.append((kt_n - 8 + n0, lambda n0=n0: qk_chain("q", 0, n0)))

        # ------------- attention: flat software pipeline over blocks -------------
        # block b = (hp, qc); global period t: QK/ACT of block t//16 period
        # t%16, PV of the block two periods back.  Blocks overlap so the
        # scalar engine (exp) never waits at block seams.
        blocks = [(hp, qc) for hp in range(HPN) for qc in range(qcn)]
        NB = len(blocks)
        state = {}
        fins = {}

        def emit_qk(b, kt):
            hp, qc = blocks[b]
            if kt == 0:
                state[b] = (o_pool.tile([P, QC], F32, tag="o", name=f"o{b}"),
                            es_pool.tile([P, 2 * QC], F16, tag="es", name=f"es{b}"), {})
                if hp + 1 < HPN and qc == 0:
                    dl = qcn * kt_n * (hp + 1) - 8
                    for n0 in range(s // QC):
                        extra.append((dl, lambda n0=n0, c=hp + 1: qk_chain("q", c, n0)))
                        extra.append((dl, lambda n0=n0, c=hp + 1: qk_chain("k", c, n0)))
            _, _, e_tiles = state[b]
            sp = sp_pool.tile([P, 2 * QC], F32, tag="sp", name="sp")
            q0 = hp * s + qc * QC
            nc.tensor.matmul(
                sp[:, 0:QC],
                lhsT=xk_sb[0:HD, hp * s + kt * P: hp * s + (kt + 1) * P],
                rhs=xq_sb[0:HD, q0:q0 + QC], start=True, stop=True)
            nc.tensor.matmul(
                sp[:, QC:2 * QC],
                lhsT=xk_sb[HD:P, hp * s + kt * P: hp * s + (kt + 1) * P],
                rhs=xq_sb[HD:P, q0:q0 + QC], start=True, stop=True)
            if with_mask:
                mt = mt_pool.tile([P, QC], F32, tag="m", name="mt")
                nc.sync.dma_start(mt[:], maskd[kt * P:(kt + 1) * P, qc * QC:(qc + 1) * QC])
                nc.vector.tensor_tensor(sp[:, 0:QC], sp[:, 0:QC], mt[:], ALU.add)
                nc.vector.tensor_tensor(sp[:, QC:2 * QC], sp[:, QC:2 * QC], mt[:], ALU.add)
            e = e_pool.tile([P, 2 * QC], F16, tag="e", name="e")
            e_tiles[kt] = e
            nc.scalar.activation(e[:], sp[:], AF.Exp)

        def emit_pv(b, kt):
            hp, qc = blocks[b]
            o, es, e_tiles = state[b]
            e = e_tiles.pop(kt)
            xva = xv_sb[:, kt * DL + 2 * hp * HD: kt * DL + (2 * hp + 1) * HD]
            xvb = xv_sb[:, kt * DL + (2 * hp + 1) * HD: kt * DL + (2 * hp + 2) * HD]
            nc.tensor.matmul(o[0:HD, :], lhsT=xva, rhs=e[:, 0:QC],
                             start=(kt == 0), stop=(kt == kt_n - 1), skip_group_check=True)
            nc.tensor.matmul(o[HD:P, :], lhsT=xvb, rhs=e[:, QC:2 * QC],
                             start=(kt == 0), stop=(kt == kt_n - 1), skip_group_check=True)
            if kt == 0:
                nc.vector.tensor_copy(es[:], e[:])
            else:
                nc.vector.tensor_tensor(es[:], es[:], e[:], ALU.add)

        def finalize(b):
            # denominators + normalization; only base-partition-0 reciprocal /
            # broadcast patterns (nonzero-base variants stomp partition 0 of
            # unrelated SBUF on HW -- observed, not modeled by CoreSim)
            hp, qc = blocks[b]
            o, es, _ = state.pop(b)
            dn = ps_pool.tile([P, QC], F32, tag="ps", name="dn")
            nc.tensor.matmul(dn[0:1, :], lhsT=ones_sb[:, 0:1], rhs=es[:, 0:QC],
                             start=True, stop=True, skip_group_check=True)
            dn2 = ps_pool.tile([P, QC], F32, tag="ps", name="dn2")
            nc.tensor.matmul(dn2[0:1, :], lhsT=ones_sb[:, 0:1], rhs=es[:, QC:2 * QC],
                             start=True, stop=True, skip_group_check=True)
            rec = n_pool.tile([1, QC], F32, tag="r", name="rec")
            nc.vector.reciprocal_approx_fast(out=rec[:], in_=dn[0:1, :])
            rec2 = n_pool.tile([1, QC], F32, tag="r2", name="rec2")
            nc.vector.reciprocal_approx_fast(out=rec2[:], in_=dn2[0:1, :])
            bc = n_pool.tile([P, QC], F32, tag="b", name="bc")
            bc2 = n_pool.tile([HD, QC], F32, tag="b2", name="bc2")
            nc.gpsimd.partition_broadcast(bc[0:HD, :], rec[:])
            nc.gpsimd.partition_broadcast(bc2[:], rec2[:])
            nc.sync.dma_start(bc[HD:P, :], bc2[:])
            nc.vector.tensor_tensor(
                ao_sb[:, hp * s + qc * QC: hp * s + (qc + 1) * QC], o[:], bc[:], ALU.mult)

        for t in range(NB * kt_n + 2):
            b_qk, p_qk = divmod(t, kt_n)
            if b_qk < NB:
                emit_qk(b_qk, p_qk)
                if p_qk == 2 and b_qk >= 1:
                    finalize(b_qk - 1)
                    hp, qc = blocks[b_qk]
                    if hp == HPN - 1 and qc > 0:
                        dl = kt_n * (b_qk + 1) + kt_n - 6
                        for qt in range(QC // P):
                            for dh in range(D // QC):
                                extra.append(
                                    (dl, lambda qc=qc - 1, qt=qt, dh=dh: outproj_chain(qc, qt, dh)))
            b_pv, p_pv = divmod(t - 2, kt_n)
            if 0 <= b_pv < NB and t - 2 >= 0:
                emit_pv(b_pv, p_pv)
            # drain: forced when a chain's deadline nears, else paced 1-in-3
            if extra and (extra[0][0] <= t + 2 or t % 2 == 0):
                extra.popleft()[1]()
        finalize(NB - 1)
        while extra:
            extra.popleft()[1]()
        # final output projection chunk
        for qt in range(QC // P):
            for dh in range(D // QC):
                outproj_chain(qcn - 1, qt, dh)

        if dump:
            dumps = [("dxq", xq_sb), ("dxk", xk_sb), ("dxv", xv_sb), ("dao", ao_sb),
                     ("dwq", wq_sb), ("dqT", qT)]
            for nm, t in dumps:
                dd = nc.dram_tensor(nm, list(t.shape), F16, kind="ExternalOutput").ap()
                nc.sync.dma_start(dd[:, :], t[:])

    nc.compile()
    return nc


_programs = {}


def _get_program(with_mask):
    key = bool(with_mask)
    if key not in _programs:
        _programs[key] = build_program(S, with_mask=key)
    return _programs[key]


def kernel(q, k, v, mask, wq, wk, wv, wo):
    q, k, v, mask = (np.asarray(x, np.float32) for x in (q, k, v, mask))
    wq, wk, wv, wo = (np.asarray(x, np.float32) for x in (wq, wk, wv, wo))
    B = q.shape[0]
    f16 = np.float16
    qh, kh, vh = q.astype(f16), k.astype(f16), v.astype(f16)
    wqh = (wq * (1.0 / np.sqrt(HD))).astype(f16)  # fold 1/sqrt(head_dim)
    wkh, wvh, woh = wk.astype(f16), wv.astype(f16), wo.astype(f16)

    with_mask = bool(np.any(mask))
    nc = _get_program(with_mask)

    in_maps = []
    for c in range(8):
        b, g = c // 2, c % 2
        dsl = slice(g * DL, (g + 1) * DL)
        m = {
            "q": np.ascontiguousarray(qh[b]),
            "k": np.ascontiguousarray(kh[b]),
            "v": np.ascontiguousarray(vh[b]),
            "wq": np.ascontiguousarray(wqh[:, dsl]),
            "wk": np.ascontiguousarray(wkh[:, dsl]),
            "wv": np.ascontiguousarray(wvh[:, dsl]),
            "wo": np.ascontiguousarray(woh[dsl, :]),
        }
        if with_mask:
            m["maskT"] = np.ascontiguousarray(mask.reshape(S, S).T)
        in_maps.append(m)

    res = run_bass_kernel_spmd(nc, in_maps, core_ids=list(range(8))).results
    global _last_results
    _last_results = res
    out = np.empty((B, S, D), np.float32)
    for b in range(B):
        out[b] = res[2 * b]["out"] + res[2 * b + 1]["out"]
    return out


_last_results = None
